# revision 13
# baseline (speedup 1.0000x reference)
"""AttentionBlockWithSkipConnection Trainium2 kernel.

Full inputs -> full output. Data-parallel over batch B=8 across 8 cores.
Each core computes one batch: GroupNorm -> qkv 1x1conv -> full 4096x4096
attention -> proj 1x1conv -> skip add.

Layout strategy: channel-major ("transposed") throughout the middle of the
pipeline so every matmul contracts over the partition dim with no transposes
of the big attention matrix:
  h^T [C, N]          (C=256 as 2 partition-chunks of 128)
  qkv^T = w_qkv.T @ h^T               (w_qkv already stored [C, 3C] = lhsT)
  logits^T[k,q] = (K^T).T @ Q^T       (both operands channel-major)
  expT = exp(logits^T / 16)           (softmax denominator = partition sums,
                                       accumulated on DVE + ones-matmul)
  o_un^T = V.T @ expT                  (V token-major via 64 PE transposes)
  proj_un^T = w_proj.T @ o_un^T
  out = transpose(proj_un^T) * (1/colsum) + b_proj + x   (scale folded into
                                       the per-partition ACT copy after the
                                       PE transpose back to token-major)
"""

import numpy as np

import concourse.bacc as bacc
import concourse.bass as bass
import concourse.mybir as mybir
import concourse.tile as tile
from concourse.bass_utils import run_bass_kernel_spmd
from concourse.masks import make_identity

N_CORES = 8
B, H, W, C = 8, 64, 64, 256
N = H * W  # 4096 tokens
G = 32  # groups
GS = C // G  # 8 channels per group
EPS = 1e-5
CC = C // 128  # 2 channel chunks
QT = 512  # q tile (free dim of logits/attnv matmuls)
NQ = N // QT  # 8
NK = N // 128  # 32 k tiles
F32 = mybir.dt.float32

# Matmul input dtype for the heavy matmuls. float32r streams at ~4x the rate
# of float32 on the PE for free dims >= 256 (reduced internal precision).
USE_F32R = True


def _mm(ap):
    if USE_F32R:
        return ap.bitcast(mybir.dt.float32r)
    return ap


def _rw(ap):
    """Round-on-write: engine writes through this AP round to fp32r, which
    the walrus verifier requires for anything consumed by an fp32r matmul."""
    if USE_F32R:
        return ap.bitcast(mybir.dt.float32r)
    return ap


def _build():
    nc = bacc.Bacc(
        "TRN2",
        target_bir_lowering=False,
        debug=False,
        enable_asserts=True,
        num_devices=N_CORES,
    )
    x_d = nc.dram_tensor("x", [N, C], F32, kind="ExternalInput")
    gns_d = nc.dram_tensor("gn_scale", [C], F32, kind="ExternalInput")
    gnb_d = nc.dram_tensor("gn_bias", [C], F32, kind="ExternalInput")
    wq_d = nc.dram_tensor("w_qkv", [C, 3 * C], F32, kind="ExternalInput")
    bq_d = nc.dram_tensor("b_qkv", [3 * C], F32, kind="ExternalInput")
    wp_d = nc.dram_tensor("w_proj", [C, C], F32, kind="ExternalInput")
    bp_d = nc.dram_tensor("b_proj", [C], F32, kind="ExternalInput")
    out_d = nc.dram_tensor("out", [N, C], F32, kind="ExternalOutput")

    # group-aggregation masks: gA averages 8 consecutive partitions into one
    # group row; gB broadcasts group rows back to their 128 channels.
    gA_np = np.zeros((128, 16), np.float32)
    gB_np = np.zeros((16, 128), np.float32)
    for p in range(128):
        gA_np[p, p // GS] = 1.0 / GS
        gB_np[p // GS, p] = 1.0
    gA_d = nc.inline_tensor(gA_np, "gA")
    gB_d = nc.inline_tensor(gB_np, "gB")

    with tile.TileContext(nc) as tc:
        _body(tc, x_d, gns_d, gnb_d, wq_d, bq_d, wp_d, bp_d, out_d, gA_d, gB_d)
    nc.compile()
    return nc


def _body(tc, x_d, gns_d, gnb_d, wq_d, bq_d, wp_d, bp_d, out_d, gA_d, gB_d):
    nc = tc.nc
    x_tok = x_d.ap().rearrange("(nt p) c -> p nt c", p=128)  # [128, 32, 256]
    out_tok = out_d.ap().rearrange("(nt p) c -> p nt c", p=128)

    with (
        tc.tile_pool(name="consts", bufs=1) as consts,
        tc.tile_pool(name="psum_tr", bufs=2, space="PSUM") as psum_tr,
        tc.tile_pool(name="psum_mm", bufs=2, space="PSUM") as psum_mm,
        tc.tile_pool(name="psum_acc", bufs=2, space="PSUM") as psum_acc,
        tc.tile_pool(name="dram_scratch", bufs=2, space="DRAM") as dram_scratch,
        tc.tile_pool(name="qkvT", bufs=1) as qkvT_pool,
    ):
        # ---- constants ----
        ident = consts.tile([128, 128], F32)
        make_identity(nc, ident)
        ones_col = consts.tile([128, 1], F32)
        nc.vector.memset(ones_col, 1.0)
        eps_col = consts.tile([128, 1], F32)
        nc.vector.memset(eps_col, EPS)
        gA = consts.tile([128, 16], F32)
        nc.sync.dma_start(out=gA, in_=gA_d.ap())
        gB = consts.tile([16, 128], F32)
        nc.sync.dma_start(out=gB, in_=gB_d.ap())
        wq_stage = consts.tile([128, CC, 3 * C], F32)
        nc.sync.dma_start(
            out=wq_stage, in_=wq_d.ap().rearrange("(cc p) d -> p cc d", p=128)
        )
        wq = consts.tile([128, CC, 3 * C], F32)
        nc.vector.tensor_copy(out=_rw(wq), in_=wq_stage)
        wp_stage = consts.tile([128, CC, C], F32)
        nc.sync.dma_start(
            out=wp_stage, in_=wp_d.ap().rearrange("(cc p) d -> p cc d", p=128)
        )
        wp = consts.tile([128, CC, C], F32)
        nc.vector.tensor_copy(out=_rw(wp), in_=wp_stage)
        bq = consts.tile([128, 6], F32)
        nc.sync.dma_start(out=bq, in_=bq_d.ap().rearrange("(m p) -> p m", p=128))
        bp_rep = consts.tile([128, C], F32)
        nc.sync.dma_start(
            out=bp_rep,
            in_=bass.AP(tensor=bp_d, offset=0, ap=[[0, 128], [1, C]]),
        )
        gns = consts.tile([128, CC], F32)
        nc.sync.dma_start(out=gns, in_=gns_d.ap().rearrange("(cc p) -> p cc", p=128))
        gnb = consts.tile([128, CC], F32)
        nc.sync.dma_start(out=gnb, in_=gnb_d.ap().rearrange("(cc p) -> p cc", p=128))

        qkvT = qkvT_pool.tile([128, 6, N], F32)  # 96KB/partition

        # ---- phase A: load x, transpose to channel-major, groupnorm ----
        with (
            tc.tile_pool(name="xcm", bufs=1) as xcm_pool,
            tc.tile_pool(name="xtm", bufs=1) as xtm_pool,
            tc.tile_pool(name="gn_stats", bufs=2) as gn_stats,
        ):
            x_cm = xcm_pool.tile([128, CC, N], F32)  # 32KB/partition
            x_tm = xtm_pool.tile([128, 32, C], F32)  # 32KB/partition
            for dchunk in range(4):
                nc.sync.dma_start(
                    out=x_tm[:, dchunk * 8 : (dchunk + 1) * 8, :],
                    in_=x_tok[:, dchunk * 8 : (dchunk + 1) * 8, :],
                )
            for nt in range(32):
                for cc in range(CC):
                    ps = psum_tr.tile([128, 128], F32, tag="tr")
                    nc.tensor.transpose(
                        ps, x_tm[:, nt, cc * 128 : (cc + 1) * 128], ident
                    )
                    nc.vector.tensor_copy(
                        out=_rw(x_cm[:, cc, nt * 128 : (nt + 1) * 128]), in_=ps
                    )

            # per-channel stats over the 4096 tokens
            ab = gn_stats.tile([128, CC, 2], F32)  # (a, b) per channel
            for cc in range(CC):
                stats = gn_stats.tile([128, 8, 6], F32, tag="stats")
                xg = x_cm[:, cc, :].rearrange("p (s f) -> p s f", f=512)
                for s in range(8):
                    nc.vector.bn_stats(out=stats[:, s, :], in_=xg[:, s, :])
                mv = gn_stats.tile([128, 2], F32, tag="mv")
                nc.vector.bn_aggr(out=mv, in_=stats)
                # mv2 = (mean, E[x^2])
                mv2 = gn_stats.tile([128, 2], F32, tag="mv2")
                nc.vector.tensor_copy(out=mv2[:, 0:1], in_=mv[:, 0:1])
                nc.vector.tensor_mul(out=mv2[:, 1:2], in0=mv[:, 0:1], in1=mv[:, 0:1])
                nc.vector.tensor_add(out=mv2[:, 1:2], in0=mv2[:, 1:2], in1=mv[:, 1:2])
                # aggregate to 16 group rows, then broadcast back to channels
                gp = psum_tr.tile([16, 2], F32, tag="tr", name="gp")
                nc.tensor.matmul(gp, lhsT=gA, rhs=mv2, start=True, stop=True)
                gp_sb = gn_stats.tile([16, 2], F32, tag="gp_sb")
                nc.vector.tensor_copy(out=gp_sb, in_=gp)
                chs = psum_tr.tile([128, 2], F32, tag="tr", name="chs")
                nc.tensor.matmul(chs, lhsT=gB, rhs=gp_sb, start=True, stop=True)
                chs_sb = gn_stats.tile([128, 2], F32, tag="chs_sb")
                nc.vector.tensor_copy(out=chs_sb, in_=chs)
                # var = E[x^2] - mean^2 ; rstd = 1/sqrt(var+eps)
                var = gn_stats.tile([128, 1], F32, tag="var")
                msq = gn_stats.tile([128, 1], F32, tag="msq")
                nc.vector.tensor_mul(out=msq, in0=chs_sb[:, 0:1], in1=chs_sb[:, 0:1])
                nc.vector.tensor_sub(out=var, in0=chs_sb[:, 1:2], in1=msq)
                nc.scalar.activation(
                    out=var,
                    in_=var,
                    func=mybir.ActivationFunctionType.Sqrt,
                    bias=eps_col,
                )
                rstd = gn_stats.tile([128, 1], F32, tag="rstd")
                nc.vector.reciprocal(out=rstd, in_=var)
                # a = rstd*gn_scale ; b = gn_bias - mean*a
                nc.vector.tensor_mul(
                    out=ab[:, cc, 0:1], in0=rstd, in1=gns[:, cc : cc + 1]
                )
                nc.vector.tensor_mul(out=msq, in0=chs_sb[:, 0:1], in1=ab[:, cc, 0:1])
                nc.vector.tensor_sub(
                    out=ab[:, cc, 1:2], in0=gnb[:, cc : cc + 1], in1=msq
                )
            # normalize in place: h = x*a + b
            for cc in range(CC):
                nc.vector.tensor_scalar(
                    out=_rw(x_cm[:, cc, :]),
                    in0=x_cm[:, cc, :],
                    scalar1=ab[:, cc, 0:1],
                    scalar2=ab[:, cc, 1:2],
                    op0=mybir.AluOpType.mult,
                    op1=mybir.AluOpType.add,
                )

            # ---- phase B: qkv^T = w_qkv.T @ h^T  (+ b_qkv) ----
            for m in range(6):
                for qt in range(NQ):
                    ps = psum_mm.tile([128, QT], F32, tag="mm")
                    for cc in range(CC):
                        nc.tensor.matmul(
                            ps,
                            lhsT=_mm(wq[:, cc, m * 128 : (m + 1) * 128]),
                            rhs=_mm(x_cm[:, cc, qt * QT : (qt + 1) * QT]),
                            start=(cc == 0),
                            stop=(cc == CC - 1),
                        )
                    nc.scalar.activation(
                        out=_rw(qkvT[:, m, qt * QT : (qt + 1) * QT]),
                        in_=ps,
                        func=mybir.ActivationFunctionType.Identity,
                        bias=bq[:, m : m + 1],
                    )

        # ---- phase C: V token-major via PE transposes ----
        with tc.tile_pool(name="vtm", bufs=1) as vtm_pool:
            v_tm = vtm_pool.tile([128, 32, C], F32)
            for nt in range(32):
                for cc in range(CC):
                    ps = psum_tr.tile([128, 128], F32, tag="tr")
                    nc.tensor.transpose(
                        ps, qkvT[:, 4 + cc, nt * 128 : (nt + 1) * 128], ident
                    )
                    nc.vector.tensor_copy(
                        out=_rw(v_tm[:, nt, cc * 128 : (cc + 1) * 128]), in_=ps
                    )

            # ---- phase D: attention + proj + skip, per q tile ----
            with (
                tc.tile_pool(name="expp", bufs=3) as expp,
                tc.tile_pool(name="accp", bufs=2) as accp,
                tc.tile_pool(name="owork", bufs=2) as owork,
            ):
                for qt in range(NQ):
                    av_ps = [
                        psum_acc.tile(
                            [128, QT], F32, tag=f"av_ps{cc}", name=f"av_ps{cc}"
                        )
                        for cc in range(CC)
                    ]
                    expacc = accp.tile([128, QT], F32, tag="expacc")

                    def emit_lg(kt):
                        lg = psum_mm.tile([128, QT], F32, tag="mm", name="lg")
                        for cc in range(CC):
                            nc.tensor.matmul(
                                lg,
                                lhsT=_mm(qkvT[:, 2 + cc, kt * 128 : (kt + 1) * 128]),
                                rhs=_mm(qkvT[:, cc, qt * QT : (qt + 1) * QT]),
                                start=(cc == 0),
                                stop=(cc == CC - 1),
                            )
                        return lg

                    # software pipeline: keep the PE busy on logits(kt+1)
                    # while ACT computes exp(kt), so av(kt) never stalls PE.
                    lg = emit_lg(0)
                    for kt in range(NK):
                        expT = expp.tile([128, QT], F32, tag="expT")
                        nc.scalar.activation(
                            out=_rw(expT),
                            in_=lg,
                            func=mybir.ActivationFunctionType.Exp,
                            scale=1.0 / 16.0,
                        )
                        if kt + 1 < NK:
                            lg = emit_lg(kt + 1)
                        if kt == 0:
                            nc.vector.tensor_copy(out=expacc, in_=expT)
                        else:
                            nc.vector.tensor_add(out=expacc, in0=expacc, in1=expT)
                        for cc in range(CC):
                            nc.tensor.matmul(
                                av_ps[cc],
                                lhsT=_mm(v_tm[:, kt, cc * 128 : (cc + 1) * 128]),
                                rhs=_mm(expT),
                                start=(kt == 0),
                                stop=(kt == NK - 1),
                            )
                    # softmax denominator: column sums of expT = ones @ expacc
                    cs = psum_tr.tile([1, QT], F32, tag="tr", name="cs")
                    nc.tensor.matmul(cs, lhsT=ones_col, rhs=expacc, start=True, stop=True)
                    cs_sb = owork.tile([1, QT], F32, tag="cs_sb")
                    nc.vector.tensor_copy(out=cs_sb, in_=cs)
                    cs_dram = dram_scratch.tile([QT], F32)
                    nc.sync.dma_start(out=cs_dram, in_=cs_sb)
                    recip = owork.tile([128, 4], F32, tag="recip")
                    nc.sync.dma_start(
                        out=recip, in_=cs_dram.rearrange("(qq p) -> p qq", p=128)
                    )
                    nc.vector.reciprocal(out=recip, in_=recip)

                    # proj_un^T = w_proj.T @ o_un^T
                    av_sb = owork.tile([128, CC, QT], F32, tag="av_sb")
                    for cc in range(CC):
                        nc.scalar.copy(out=_rw(av_sb[:, cc, :]), in_=av_ps[cc])
                    pj_sb = owork.tile([128, CC, QT], F32, tag="pj_sb")
                    for dc in range(CC):
                        ps = psum_mm.tile([128, QT], F32, tag="mm", name="pj_ps")
                        for cc in range(CC):
                            nc.tensor.matmul(
                                ps,
                                lhsT=_mm(wp[:, cc, dc * 128 : (dc + 1) * 128]),
                                rhs=_mm(av_sb[:, cc, :]),
                                start=(cc == 0),
                                stop=(cc == CC - 1),
                            )
                        nc.scalar.copy(out=pj_sb[:, dc, :], in_=ps)

                    # back to token-major; fold 1/colsum into the copy scale
                    out_sb = owork.tile([128, 4, C], F32, tag="out_sb")
                    x_re = owork.tile([128, 4, C], F32, tag="x_re")
                    nc.sync.dma_start(
                        out=x_re, in_=x_tok[:, qt * 4 : (qt + 1) * 4, :]
                    )
                    for qq in range(4):
                        for dc in range(CC):
                            ps = psum_tr.tile([128, 128], F32, tag="tr", name="ps_out")
                            nc.tensor.transpose(
                                ps, pj_sb[:, dc, qq * 128 : (qq + 1) * 128], ident
                            )
                            nc.scalar.activation(
                                out=out_sb[:, qq, dc * 128 : (dc + 1) * 128],
                                in_=ps,
                                func=mybir.ActivationFunctionType.Copy,
                                scale=recip[:, qq : qq + 1],
                            )
                        nc.vector.tensor_add(
                            out=out_sb[:, qq, :], in0=out_sb[:, qq, :], in1=bp_rep
                        )
                        nc.vector.tensor_add(
                            out=out_sb[:, qq, :], in0=out_sb[:, qq, :], in1=x_re[:, qq, :]
                        )
                    nc.sync.dma_start(
                        out=out_tok[:, qt * 4 : (qt + 1) * 4, :], in_=out_sb
                    )


_NC = None


def _get_nc():
    global _NC
    if _NC is None:
        _NC = _build()
    return _NC


_RUNNER = None

IN_NAMES = ["x", "gn_scale", "gn_bias", "w_qkv", "b_qkv", "w_proj", "b_proj"]


def _get_runner():
    """Cached jitted shard_map executable over the 8 cores (the equivalent of
    run_bass_kernel_spmd's axon path, but built once instead of per call)."""
    global _RUNNER
    if _RUNNER is not None:
        return _RUNNER
    import jax
    import jax.numpy as jnp
    from jax.sharding import Mesh, PartitionSpec
    from jax.experimental.shard_map import shard_map
    from concourse import bass2jax

    nc = _get_nc()
    bass2jax.install_neuronx_cc_hook()

    in_names = list(IN_NAMES) + ["out"]
    if nc.partition_id_tensor is not None:
        in_names.append(nc.partition_id_tensor.name)

    def _body(*args):
        operands = list(args)
        if nc.partition_id_tensor is not None:
            operands.append(bass2jax.partition_id_tensor())
        outs = bass2jax._bass_exec_p.bind(
            *operands,
            out_avals=(jax.core.ShapedArray((N, C), np.float32),),
            in_names=tuple(in_names),
            out_names=("out",),
            lowering_input_output_aliases=(),
            sim_require_finite=True,
            sim_require_nnan=True,
            nc=nc,
        )
        return tuple(outs)

    devices = jax.devices()[:N_CORES]
    mesh = Mesh(np.asarray(devices), ("core",))
    in_specs = (PartitionSpec("core"),) * (len(IN_NAMES) + 1)
    out_specs = (PartitionSpec("core"),)
    sharded = jax.jit(
        shard_map(
            _body, mesh=mesh, in_specs=in_specs, out_specs=out_specs, check_rep=False
        ),
        donate_argnums=(len(IN_NAMES),),
        keep_unused=True,
    )
    _RUNNER = sharded
    return _RUNNER


def kernel(x, gn_scale, gn_bias, w_qkv, b_qkv, w_proj, b_proj):
    sharded = _get_runner()
    x = np.ascontiguousarray(np.asarray(x, dtype=np.float32).reshape(B * N, C))
    shared = {
        "gn_scale": np.asarray(gn_scale, np.float32),
        "gn_bias": np.asarray(gn_bias, np.float32),
        "w_qkv": np.ascontiguousarray(np.asarray(w_qkv, np.float32)),
        "b_qkv": np.asarray(b_qkv, np.float32),
        "w_proj": np.ascontiguousarray(np.asarray(w_proj, np.float32)),
        "b_proj": np.asarray(b_proj, np.float32),
    }
    # shard_map slices axis 0 across cores: x gets its own batch; the shared
    # weights are tiled 8x so every core sees an identical copy.
    concat = [x]
    for name in IN_NAMES[1:]:
        a = shared[name]
        concat.append(np.concatenate([a] * N_CORES, axis=0))
    zeros = np.zeros((N_CORES * N, C), np.float32)
    (out,) = sharded(*concat, zeros)
    return np.asarray(out).reshape(B, H, W, C)


# revision 25
# speedup vs baseline: 229.6988x; 229.6988x over previous
"""AttentionBlockWithSkipConnection Trainium2 kernel.

Full inputs -> full output. Data-parallel over batch B=8 across 8 cores.
Each core computes one batch: GroupNorm -> qkv 1x1conv -> full 4096x4096
attention -> proj 1x1conv -> skip add.

Layout strategy: channel-major ("transposed") throughout the middle of the
pipeline so every matmul contracts over the partition dim and the 4096x4096
attention matrix is never transposed or spilled:
  x^T [C, N]           (C=256 as 2 partition-chunks of 128; 64 PE transposes)
  GroupNorm folded into the qkv weights: h = a*x + b (per channel) =>
      qkv^T = (w*a)^T @ x^T + (w^T b + b_qkv)
  logits^T[k,q] = (K^T).T @ Q^T        (both operands channel-major)
  expT = exp(logits^T / 16)            (softmax denominator = partition sums,
                                        accumulated on DVE + GPSIMD, finished
                                        by a ones-row matmul)
  o_un^T = V.T @ expT                  (V token-major via 64 PE transposes,
                                        flash-style PSUM accumulation)
  proj_un^T = w_proj.T @ o_un^T + b_proj x colsum    (bias as a rank-1 matmul
                                        so it survives the deferred softmax
                                        normalization)
  out = transpose(proj_un^T) * (1/colsum) + x        (per-partition scale on
                                        the DVE after PE-transposing back to
                                        token-major)

All heavy matmuls run in float32r (~4x the fp32 rate for free dims >= 256,
~1.6e-4 relative error); producers round-on-write as walrus requires.
"""

import numpy as np

import concourse.bacc as bacc
import concourse.bass as bass
import concourse.mybir as mybir
import concourse.tile as tile

N_CORES = 8
B, H, W, C = 8, 64, 64, 256
N = H * W  # 4096 tokens
G = 32  # groups
GS = C // G  # 8 channels per group
EPS = 1e-5
CC = C // 128  # 2 channel chunks
QT = 512  # q tile (free dim of logits/attnv matmuls)
NQ = N // QT  # 8
NK = N // 128  # 32 k tiles
F32 = mybir.dt.float32

USE_F32R = True


def _mm(ap):
    """Matmul-input view: fp32 data consumed as float32r."""
    if USE_F32R:
        return ap.bitcast(mybir.dt.float32r)
    return ap


def _rw(ap):
    """Round-on-write view: engine writes through this AP round to fp32r,
    which the walrus verifier requires for fp32r matmul inputs."""
    if USE_F32R:
        return ap.bitcast(mybir.dt.float32r)
    return ap


def _build(repeat=1):
    nc = bacc.Bacc(
        "TRN2",
        target_bir_lowering=False,
        debug=False,
        enable_asserts=True,
        num_devices=N_CORES,
    )
    x_d = nc.dram_tensor("x", [N, C], F32, kind="ExternalInput")
    gns_d = nc.dram_tensor("gn_scale", [C], F32, kind="ExternalInput")
    gnb_d = nc.dram_tensor("gn_bias", [C], F32, kind="ExternalInput")
    wq_d = nc.dram_tensor("w_qkv", [C, 3 * C], F32, kind="ExternalInput")
    bq_d = nc.dram_tensor("b_qkv", [3 * C], F32, kind="ExternalInput")
    wp_d = nc.dram_tensor("w_proj", [C, C], F32, kind="ExternalInput")
    bp_d = nc.dram_tensor("b_proj", [C], F32, kind="ExternalInput")
    out_d = nc.dram_tensor("out", [N, C], F32, kind="ExternalOutput")

    # group-aggregation masks: gA averages 8 consecutive partitions into one
    # group row; gB broadcasts group rows back to their 128 channels.
    gA_np = np.zeros((128, 16), np.float32)
    gB_np = np.zeros((16, 128), np.float32)
    for p in range(128):
        gA_np[p, p // GS] = 1.0 / GS
        gB_np[p // GS, p] = 1.0
    gA_d = nc.inline_tensor(gA_np, "gA")
    gB_d = nc.inline_tensor(gB_np, "gB")
    ident_d = nc.inline_tensor(np.eye(128, dtype=np.float32), "ident")

    with tile.TileContext(nc) as tc:
        for _ in range(repeat):
            _body(tc, x_d, gns_d, gnb_d, wq_d, bq_d, wp_d, bp_d, out_d,
                  gA_d, gB_d, ident_d)
    nc.compile()
    return nc


def _body(tc, x_d, gns_d, gnb_d, wq_d, bq_d, wp_d, bp_d, out_d,
          gA_d, gB_d, ident_d):
    nc = tc.nc
    x_tok = x_d.ap().rearrange("(p nt) c -> p nt c", p=128)  # [128, 32, 256]
    out_tok = out_d.ap().rearrange("(p nt) c -> p nt c", p=128)

    with (
        tc.tile_pool(name="consts", bufs=1) as consts,
        tc.tile_pool(name="psum_tr", bufs=2, space="PSUM") as psum_tr,
        tc.tile_pool(name="psum_mm", bufs=4, space="PSUM") as psum_mm,
        tc.tile_pool(name="psum_acc", bufs=1, space="PSUM") as psum_acc,
        tc.tile_pool(name="dram_scratch", bufs=2, space="DRAM") as dram_scratch,
        tc.tile_pool(name="qkvT", bufs=1) as qkvT_pool,
    ):
        # ---- input DMAs: x first (PE transposes gate on it), identity on
        # the same fast HWDGE queue, weights on the cheap GPSIMD queue ----
        ident = consts.tile([128, 128], F32)
        nc.sync.dma_start(out=ident, in_=ident_d.ap())
        qkvT = qkvT_pool.tile([128, 6, N], F32)  # 96KB/partition

        with (
            tc.tile_pool(name="xcm", bufs=1) as xcm_pool,
            tc.tile_pool(name="xtm", bufs=1) as xtm_pool,
            tc.tile_pool(name="gn_stats", bufs=2) as gn_stats,
        ):
            x_cm = xcm_pool.tile([128, CC, N], F32)  # 32KB/partition
            x_tm = xtm_pool.tile([128, 32, C], F32)  # 32KB/partition
            dma_engs = [nc.sync, nc.scalar, nc.gpsimd]
            for dchunk in range(16):
                dma_engs[dchunk % 3].dma_start(
                    out=x_tm[:, dchunk * 2 : (dchunk + 1) * 2, :],
                    in_=x_tok[:, dchunk * 2 : (dchunk + 1) * 2, :],
                )

            # ---- weights / small constants (SWDGE, Pool queue) ----
            gA = consts.tile([128, 16], F32)
            nc.gpsimd.dma_start(out=gA, in_=gA_d.ap())
            gB = consts.tile([16, 128], F32)
            nc.gpsimd.dma_start(out=gB, in_=gB_d.ap())
            wq_stage = consts.tile([128, CC, 3 * C], F32)
            nc.gpsimd.dma_start(
                out=wq_stage, in_=wq_d.ap().rearrange("(cc p) d -> p cc d", p=128)
            )
            wp_stage = consts.tile([128, CC, C], F32)
            nc.gpsimd.dma_start(
                out=wp_stage, in_=wp_d.ap().rearrange("(cc p) d -> p cc d", p=128)
            )
            wp = consts.tile([128, CC, C], F32)
            nc.vector.tensor_copy(out=_rw(wp), in_=wp_stage)
            bq = consts.tile([128, 6], F32)
            nc.gpsimd.dma_start(
                out=bq, in_=bq_d.ap().rearrange("(m p) -> p m", p=128)
            )
            bp_row = consts.tile([1, C], F32)
            nc.gpsimd.dma_start(
                out=bp_row, in_=bp_d.ap().rearrange("(o c) -> o c", o=1)
            )
            bp_r = consts.tile([1, C], F32)
            nc.vector.tensor_copy(out=_rw(bp_r), in_=bp_row)
            gns = consts.tile([128, CC], F32)
            nc.gpsimd.dma_start(
                out=gns, in_=gns_d.ap().rearrange("(cc p) -> p cc", p=128)
            )
            gnb = consts.tile([128, CC], F32)
            nc.gpsimd.dma_start(
                out=gnb, in_=gnb_d.ap().rearrange("(cc p) -> p cc", p=128)
            )
            ones_raw = consts.tile([128, 1], F32)
            nc.vector.memset(ones_raw, 1.0)
            ones_col = consts.tile([128, 1], F32)
            nc.vector.tensor_copy(out=_rw(ones_col), in_=ones_raw)
            eps_col = consts.tile([128, 1], F32)
            nc.vector.memset(eps_col, EPS)

            # ---- phase A: transpose x to channel-major; bn_stats interleaved
            # so the statistics finish right after the last transpose ----
            stats = gn_stats.tile([128, CC, 8, 6], F32)
            for s in range(8):
                for nt in range(4 * s, 4 * s + 4):
                    for cc in range(CC):
                        ps = psum_tr.tile([128, 128], F32, tag="tr")
                        nc.tensor.transpose(
                            ps, x_tm[:, nt, cc * 128 : (cc + 1) * 128], ident
                        )
                        # alternate PSUM->SBUF copies across DVE and ACT so
                        # neither engine serializes the prologue
                        ceng = nc.vector if (nt + cc) % 2 == 0 else nc.scalar
                        if ceng is nc.vector:
                            ceng.tensor_copy(
                                out=_rw(x_cm[:, cc, nt * 128 : (nt + 1) * 128]),
                                in_=ps,
                            )
                        else:
                            nc.scalar.copy(
                                out=_rw(x_cm[:, cc, nt * 128 : (nt + 1) * 128]),
                                in_=ps,
                            )
                for cc in range(CC):
                    nc.vector.bn_stats(
                        out=stats[:, cc, s, :],
                        in_=x_cm[:, cc, s * 512 : (s + 1) * 512],
                    )

            # ---- groupnorm stats -> per-channel affine (a, b) ----
            ab = gn_stats.tile([128, CC, 2], F32)  # (a, b) per channel
            for cc in range(CC):
                mv = gn_stats.tile([128, 2], F32, tag="mv")
                nc.vector.bn_aggr(out=mv, in_=stats[:, cc, :, :])
                # mv2 = (mean, E[x^2])
                mv2 = gn_stats.tile([128, 2], F32, tag="mv2")
                nc.vector.tensor_copy(out=mv2[:, 0:1], in_=mv[:, 0:1])
                nc.vector.tensor_mul(out=mv2[:, 1:2], in0=mv[:, 0:1], in1=mv[:, 0:1])
                nc.vector.tensor_add(out=mv2[:, 1:2], in0=mv2[:, 1:2], in1=mv[:, 1:2])
                # aggregate to 16 group rows, then broadcast back to channels
                gp = psum_tr.tile([16, 2], F32, tag="tr", name="gp")
                nc.tensor.matmul(gp, lhsT=gA, rhs=mv2, start=True, stop=True)
                gp_sb = gn_stats.tile([16, 2], F32, tag="gp_sb")
                nc.vector.tensor_copy(out=gp_sb, in_=gp)
                chs = psum_tr.tile([128, 2], F32, tag="tr", name="chs")
                nc.tensor.matmul(chs, lhsT=gB, rhs=gp_sb, start=True, stop=True)
                chs_sb = gn_stats.tile([128, 2], F32, tag="chs_sb")
                nc.vector.tensor_copy(out=chs_sb, in_=chs)
                # var = E[x^2] - mean^2 ; rstd = 1/sqrt(var+eps)
                var = gn_stats.tile([128, 1], F32, tag="var")
                msq = gn_stats.tile([128, 1], F32, tag="msq")
                nc.vector.tensor_mul(out=msq, in0=chs_sb[:, 0:1], in1=chs_sb[:, 0:1])
                nc.vector.tensor_sub(out=var, in0=chs_sb[:, 1:2], in1=msq)
                nc.scalar.activation(
                    out=var,
                    in_=var,
                    func=mybir.ActivationFunctionType.Sqrt,
                    bias=eps_col,
                )
                rstd = gn_stats.tile([128, 1], F32, tag="rstd")
                nc.vector.reciprocal(out=rstd, in_=var)
                # a = rstd*gn_scale ; b = gn_bias - mean*a
                nc.vector.tensor_mul(
                    out=ab[:, cc, 0:1], in0=rstd, in1=gns[:, cc : cc + 1]
                )
                nc.vector.tensor_mul(out=msq, in0=chs_sb[:, 0:1], in1=ab[:, cc, 0:1])
                nc.vector.tensor_sub(
                    out=ab[:, cc, 1:2], in0=gnb[:, cc : cc + 1], in1=msq
                )

            # ---- fold the affine into the qkv weights:
            # qkv^T = (w*a)^T x^T + (w^T b + b_qkv) ----
            wq = consts.tile([128, CC, 3 * C], F32)
            for m in range(6):
                for cc in range(CC):
                    nc.scalar.mul(
                        out=_rw(wq[:, cc, m * 128 : (m + 1) * 128]),
                        in_=wq_stage[:, cc, m * 128 : (m + 1) * 128],
                        mul=ab[:, cc, 0:1],
                    )
            bias2 = gn_stats.tile([128, 6], F32)
            for m in range(6):
                psb = psum_tr.tile([128, 1], F32, tag="tr", name="psb")
                for cc in range(CC):
                    nc.tensor.matmul(
                        psb,
                        lhsT=wq_stage[:, cc, m * 128 : (m + 1) * 128],
                        rhs=ab[:, cc, 1:2],
                        start=(cc == 0),
                        stop=(cc == CC - 1),
                    )
                nc.vector.tensor_add(
                    out=bias2[:, m : m + 1], in0=psb, in1=bq[:, m : m + 1]
                )

            # ---- phase B: qkv^T = wq.T @ x^T (+ bias2) ----
            for m in range(6):
                for qt in range(NQ):
                    ps = psum_mm.tile([128, QT], F32, tag="mm")
                    for cc in range(CC):
                        nc.tensor.matmul(
                            ps,
                            lhsT=_mm(wq[:, cc, m * 128 : (m + 1) * 128]),
                            rhs=_mm(x_cm[:, cc, qt * QT : (qt + 1) * QT]),
                            start=(cc == 0),
                            stop=(cc == CC - 1),
                        )
                    if qt % 2 == 0:
                        nc.scalar.activation(
                            out=_rw(qkvT[:, m, qt * QT : (qt + 1) * QT]),
                            in_=ps,
                            func=mybir.ActivationFunctionType.Identity,
                            bias=bias2[:, m : m + 1],
                        )
                    else:
                        nc.vector.tensor_scalar_add(
                            out=_rw(qkvT[:, m, qt * QT : (qt + 1) * QT]),
                            in0=ps,
                            scalar1=bias2[:, m : m + 1],
                        )

        # ---- phase C: V token-major via PE transposes ----
        with tc.tile_pool(name="vtm", bufs=1) as vtm_pool:
            v_tm = vtm_pool.tile([128, 32, C], F32)
            for nt in range(32):
                for cc in range(CC):
                    ps = psum_tr.tile([128, 128], F32, tag="tr")
                    nc.tensor.transpose(
                        ps, qkvT[:, 4 + cc, nt * 128 : (nt + 1) * 128], ident
                    )
                    nc.vector.tensor_copy(
                        out=_rw(v_tm[:, nt, cc * 128 : (cc + 1) * 128]), in_=ps
                    )

            # ---- phase D: attention + proj + skip, per q tile ----
            with (
                tc.tile_pool(name="expp", bufs=6) as expp,
                tc.tile_pool(name="accp", bufs=2) as accp,
                tc.tile_pool(name="owork", bufs=2) as owork,
            ):
                def emit_lg(qt, kt):
                    lg = psum_mm.tile([128, QT], F32, tag="mm", name="lg")
                    for cc in range(CC):
                        nc.tensor.matmul(
                            lg,
                            lhsT=_mm(qkvT[:, 2 + cc, kt * 128 : (kt + 1) * 128]),
                            rhs=_mm(qkvT[:, cc, qt * QT : (qt + 1) * QT]),
                            start=(cc == 0),
                            stop=(cc == CC - 1),
                        )
                    return lg

                # logits tiles prefetched across the qt boundary
                next_lgs = {kk: emit_lg(0, kk) for kk in range(2)}
                for qt in range(NQ):
                    av_ps = [
                        psum_acc.tile(
                            [128, QT], F32, tag=f"av_ps{cc}", name=f"av_ps{cc}"
                        )
                        for cc in range(CC)
                    ]
                    expacc = accp.tile([128, QT], F32, tag="expacc")
                    expacc2 = accp.tile([128, QT], F32, tag="expacc2")

                    def emit_av(kt, expT):
                        for cc in range(CC):
                            nc.tensor.matmul(
                                av_ps[cc],
                                lhsT=_mm(v_tm[:, kt, cc * 128 : (cc + 1) * 128]),
                                rhs=_mm(expT),
                                start=(kt == 0),
                                stop=(kt == NK - 1),
                            )

                    # software pipeline, kt unrolled by 2: the PE stays 4+
                    # logits-matmuls ahead of each av, fully hiding the
                    # lg -> exp(ACT) -> av semaphore+latency chain (~910ns).
                    lgs = next_lgs
                    lgs[2] = emit_lg(qt, 2)
                    lgs[3] = emit_lg(qt, 3)
                    for kt0 in range(0, NK, 2):
                        expTs = {}
                        for j in (kt0, kt0 + 1):
                            lg = lgs.pop(j)
                            expT = expp.tile([128, QT], F32, tag="expT")
                            nc.scalar.activation(
                                out=_rw(expT),
                                in_=lg,
                                func=mybir.ActivationFunctionType.Exp,
                                scale=1.0 / 16.0,
                            )
                            expTs[j] = expT
                            # softmax-denominator accumulation, split across
                            # DVE and the otherwise-idle GPSIMD engine
                            eng = nc.gpsimd if j % 2 == 0 else nc.vector
                            acc = expacc2 if j % 2 == 0 else expacc
                            if j < 2:
                                eng.tensor_copy(out=acc, in_=expT)
                            else:
                                eng.tensor_add(out=acc, in0=acc, in1=expT)
                        for j in (kt0 + 4, kt0 + 5):
                            if j < NK:
                                lgs[j] = emit_lg(qt, j)
                        for j in (kt0, kt0 + 1):
                            emit_av(j, expTs[j])

                    # prefetch the next q tile's first logits so the PE
                    # stays busy while the colsum/proj chain drains
                    if qt + 1 < NQ:
                        next_lgs = {kk: emit_lg(qt + 1, kk) for kk in range(2)}
                    expcomb = accp.tile([128, QT], F32, tag="expcomb")
                    nc.vector.tensor_add(
                        out=_rw(expcomb), in0=expacc, in1=expacc2
                    )
                    # softmax denominator: colsum = ones.T @ expcomb
                    cs = psum_tr.tile([1, QT], F32, tag="tr", name="cs")
                    nc.tensor.matmul(
                        cs, lhsT=_mm(ones_col), rhs=_mm(expcomb), start=True, stop=True
                    )
                    cs_sb = owork.tile([1, QT], F32, tag="cs_sb")
                    nc.vector.tensor_copy(out=_rw(cs_sb), in_=cs)
                    cs_dram = dram_scratch.tile([QT], F32)
                    nc.sync.dma_start(out=cs_dram, in_=cs_sb)
                    recip = owork.tile([128, 4], F32, tag="recip")
                    nc.sync.dma_start(
                        out=recip, in_=cs_dram.rearrange("(qq p) -> p qq", p=128)
                    )
                    nc.vector.reciprocal(out=recip, in_=recip)

                    # proj_un^T = w_proj.T @ o_un^T + b_proj x colsum
                    av_sb = owork.tile([128, CC, QT], F32, tag="av_sb")
                    for cc in range(CC):
                        nc.vector.tensor_copy(out=_rw(av_sb[:, cc, :]), in_=av_ps[cc])
                    pj_sb = owork.tile([128, CC, QT], F32, tag="pj_sb")
                    for dc in range(CC):
                        ps = psum_mm.tile([128, QT], F32, tag="mm", name="pj_ps")
                        for cc in range(CC):
                            nc.tensor.matmul(
                                ps,
                                lhsT=_mm(wp[:, cc, dc * 128 : (dc + 1) * 128]),
                                rhs=_mm(av_sb[:, cc, :]),
                                start=(cc == 0),
                                stop=False,
                            )
                        nc.tensor.matmul(
                            ps,
                            lhsT=_mm(bp_r[:, dc * 128 : (dc + 1) * 128]),
                            rhs=_mm(cs_sb),
                            start=False,
                            stop=True,
                        )
                        nc.scalar.copy(out=pj_sb[:, dc, :], in_=ps)

                    # back to token-major; apply 1/colsum; add skip
                    out_sb = owork.tile([128, 4, C], F32, tag="out_sb")
                    x_re = owork.tile([128, 4, C], F32, tag="x_re")
                    nc.sync.dma_start(
                        out=x_re, in_=x_tok[:, qt * 4 : (qt + 1) * 4, :]
                    )
                    for qq in range(4):
                        for dc in range(CC):
                            ps = psum_tr.tile([128, 128], F32, tag="tr", name="ps_out")
                            nc.tensor.transpose(
                                ps, pj_sb[:, dc, qq * 128 : (qq + 1) * 128], ident
                            )
                            nc.vector.tensor_scalar_mul(
                                out=out_sb[:, qq, dc * 128 : (dc + 1) * 128],
                                in0=ps,
                                scalar1=recip[:, qq : qq + 1],
                            )
                    nc.vector.tensor_add(out=out_sb, in0=out_sb, in1=x_re)
                    nc.sync.dma_start(
                        out=out_tok[:, qt * 4 : (qt + 1) * 4, :], in_=out_sb
                    )


_NC = None


def _get_nc():
    global _NC
    if _NC is None:
        _NC = _build()
    return _NC


_RUNNER = None

IN_NAMES = ["x", "gn_scale", "gn_bias", "w_qkv", "b_qkv", "w_proj", "b_proj"]


def _get_runner():
    """Cached jitted shard_map executable over the 8 cores (the equivalent of
    run_bass_kernel_spmd's axon path, but built once instead of per call)."""
    global _RUNNER
    if _RUNNER is not None:
        return _RUNNER
    import jax
    from jax.sharding import Mesh, PartitionSpec
    from jax.experimental.shard_map import shard_map
    from concourse import bass2jax

    nc = _get_nc()
    bass2jax.install_neuronx_cc_hook()

    in_names = list(IN_NAMES) + ["out"]
    if nc.partition_id_tensor is not None:
        in_names.append(nc.partition_id_tensor.name)

    def _body_fn(*args):
        operands = list(args)
        if nc.partition_id_tensor is not None:
            operands.append(bass2jax.partition_id_tensor())
        outs = bass2jax._bass_exec_p.bind(
            *operands,
            out_avals=(jax.core.ShapedArray((N, C), np.float32),),
            in_names=tuple(in_names),
            out_names=("out",),
            lowering_input_output_aliases=(),
            sim_require_finite=True,
            sim_require_nnan=True,
            nc=nc,
        )
        return tuple(outs)

    devices = jax.devices()[:N_CORES]
    mesh = Mesh(np.asarray(devices), ("core",))
    in_specs = (PartitionSpec("core"),) * (len(IN_NAMES) + 1)
    out_specs = (PartitionSpec("core"),)
    sharded = jax.jit(
        shard_map(
            _body_fn, mesh=mesh, in_specs=in_specs, out_specs=out_specs,
            check_rep=False,
        ),
        donate_argnums=(len(IN_NAMES),),
        keep_unused=True,
    )
    _RUNNER = sharded
    return _RUNNER


def kernel(x, gn_scale, gn_bias, w_qkv, b_qkv, w_proj, b_proj):
    sharded = _get_runner()
    x = np.ascontiguousarray(np.asarray(x, dtype=np.float32).reshape(B * N, C))
    shared = {
        "gn_scale": np.asarray(gn_scale, np.float32),
        "gn_bias": np.asarray(gn_bias, np.float32),
        "w_qkv": np.ascontiguousarray(np.asarray(w_qkv, np.float32)),
        "b_qkv": np.asarray(b_qkv, np.float32),
        "w_proj": np.ascontiguousarray(np.asarray(w_proj, np.float32)),
        "b_proj": np.asarray(b_proj, np.float32),
    }
    # shard_map slices axis 0 across cores: x gets its own batch; the shared
    # weights are tiled 8x so every core sees an identical copy.
    concat = [x]
    for name in IN_NAMES[1:]:
        a = shared[name]
        concat.append(np.concatenate([a] * N_CORES, axis=0))
    zeros = np.zeros((N_CORES * N, C), np.float32)
    (out,) = sharded(*concat, zeros)
    return np.asarray(out).reshape(B, H, W, C)


# revision 32
# speedup vs baseline: 6478.6568x; 28.2050x over previous
"""AttentionBlockWithSkipConnection Trainium2 kernel.

Full inputs -> full output. Data-parallel over batch B=8 across 8 cores.
Each core computes one batch: GroupNorm -> qkv 1x1conv -> full 4096x4096
attention -> proj 1x1conv -> skip add.

Layout strategy: channel-major ("transposed") throughout the middle of the
pipeline so every matmul contracts over the partition dim and the 4096x4096
attention matrix is never transposed or spilled:
  x^T [C, N]           (C=256 as 2 partition-chunks of 128; 64 PE transposes)
  GroupNorm folded into the qkv weights: h = a*x + b (per channel) =>
      qkv^T = (w*a)^T @ x^T + (w^T b + b_qkv)
  logits^T[k,q] = (K^T).T @ Q^T        (both operands channel-major)
  expT = exp(logits^T / 16)            (softmax denominator = partition sums,
                                        accumulated on DVE, finished by a
                                        ones-row matmul)
  o_un^T = V.T @ expT                  (V token-major via 64 PE transposes,
                                        flash-style PSUM accumulation)
  proj_un^T = w_proj.T @ o_un^T + b_proj x colsum    (bias as a rank-1 matmul
                                        so it survives the deferred softmax
                                        normalization)
  out = transpose(proj_un^T) * (1/colsum) + x        (per-partition scale on
                                        the DVE after PE-transposing back to
                                        token-major)

All heavy matmuls run in float32r (~4x the fp32 rate for free dims >= 256,
~1.6e-4 relative error); producers round-on-write as walrus requires.
"""

import numpy as np

import concourse.bacc as bacc
import concourse.mybir as mybir
import concourse.tile as tile

N_CORES = 8
B, H, W, C = 8, 64, 64, 256
N = H * W  # 4096 tokens
G = 32  # groups
GS = C // G  # 8 channels per group
EPS = 1e-5
CC = C // 128  # 2 channel chunks
QT = 512  # q tile (free dim of logits/attnv matmuls)
NQ = N // QT  # 8
NK = N // 128  # 32 k tiles
F32 = mybir.dt.float32

USE_F32R = True


def _mm(ap):
    """Matmul-input view: fp32 data consumed as float32r."""
    if USE_F32R:
        return ap.bitcast(mybir.dt.float32r)
    return ap


def _rw(ap):
    """Round-on-write view: engine writes through this AP round to fp32r,
    which the walrus verifier requires for fp32r matmul inputs."""
    if USE_F32R:
        return ap.bitcast(mybir.dt.float32r)
    return ap


def _build(repeat=1):
    nc = bacc.Bacc(
        "TRN2",
        target_bir_lowering=False,
        debug=False,
        enable_asserts=True,
        num_devices=N_CORES,
    )
    x_d = nc.dram_tensor("x", [N, C], F32, kind="ExternalInput")
    gns_d = nc.dram_tensor("gn_scale", [C], F32, kind="ExternalInput")
    gnb_d = nc.dram_tensor("gn_bias", [C], F32, kind="ExternalInput")
    wq_d = nc.dram_tensor("w_qkv", [C, 3 * C], F32, kind="ExternalInput")
    bq_d = nc.dram_tensor("b_qkv", [3 * C], F32, kind="ExternalInput")
    wp_d = nc.dram_tensor("w_proj", [C, C], F32, kind="ExternalInput")
    bp_d = nc.dram_tensor("b_proj", [C], F32, kind="ExternalInput")
    out_d = nc.dram_tensor("out", [N, C], F32, kind="ExternalOutput")

    # group-aggregation masks: gA averages 8 consecutive partitions into one
    # group row; gB broadcasts group rows back to their 128 channels.
    gA_np = np.zeros((128, 16), np.float32)
    gB_np = np.zeros((16, 128), np.float32)
    for p in range(128):
        gA_np[p, p // GS] = 1.0 / GS
        gB_np[p // GS, p] = 1.0
    gA_d = nc.inline_tensor(gA_np, "gA")
    gB_d = nc.inline_tensor(gB_np, "gB")
    ident_d = nc.inline_tensor(np.eye(128, dtype=np.float32), "ident")

    with tile.TileContext(nc) as tc:
        for _ in range(repeat):
            _body(tc, x_d, gns_d, gnb_d, wq_d, bq_d, wp_d, bp_d, out_d,
                  gA_d, gB_d, ident_d)
    nc.compile()
    return nc


def _body(tc, x_d, gns_d, gnb_d, wq_d, bq_d, wp_d, bp_d, out_d,
          gA_d, gB_d, ident_d):
    nc = tc.nc
    x_tok = x_d.ap().rearrange("(p nt) c -> p nt c", p=128)  # [128, 32, 256]
    out_tok = out_d.ap().rearrange("(p nt) c -> p nt c", p=128)

    with (
        tc.tile_pool(name="consts", bufs=1) as consts,
        tc.tile_pool(name="psum_tr", bufs=2, space="PSUM") as psum_tr,
        tc.tile_pool(name="psum_mm", bufs=4, space="PSUM") as psum_mm,
        tc.tile_pool(name="psum_acc", bufs=1, space="PSUM") as psum_acc,
        tc.tile_pool(name="dram_scratch", bufs=2, space="DRAM") as dram_scratch,
        tc.tile_pool(name="qkvT", bufs=1) as qkvT_pool,
    ):
        # ---- input DMAs: x first (PE transposes gate on it), identity on
        # the same fast HWDGE queue, weights on the cheap GPSIMD queue ----
        ident = consts.tile([128, 128], F32)
        nc.sync.dma_start(out=ident, in_=ident_d.ap())
        qkvT = qkvT_pool.tile([128, 6, N], F32)  # 96KB/partition

        with (
            tc.tile_pool(name="xcm", bufs=1) as xcm_pool,
            tc.tile_pool(name="xtm", bufs=1) as xtm_pool,
            tc.tile_pool(name="gn_stats", bufs=2) as gn_stats,
        ):
            x_cm = xcm_pool.tile([128, CC, N], F32)  # 32KB/partition
            x_tm = xtm_pool.tile([128, 32, C], F32)  # 32KB/partition
            dma_engs = [nc.sync, nc.scalar, nc.gpsimd]
            for dchunk in range(16):
                dma_engs[dchunk % 3].dma_start(
                    out=x_tm[:, dchunk * 2 : (dchunk + 1) * 2, :],
                    in_=x_tok[:, dchunk * 2 : (dchunk + 1) * 2, :],
                )

            # ---- weights / small constants (SWDGE, Pool queue) ----
            gA = consts.tile([128, 16], F32)
            nc.gpsimd.dma_start(out=gA, in_=gA_d.ap())
            gB = consts.tile([16, 128], F32)
            nc.gpsimd.dma_start(out=gB, in_=gB_d.ap())
            wq_stage = consts.tile([128, CC, 3 * C], F32)
            nc.gpsimd.dma_start(
                out=wq_stage, in_=wq_d.ap().rearrange("(cc p) d -> p cc d", p=128)
            )
            wp_stage = consts.tile([128, CC, C], F32)
            nc.gpsimd.dma_start(
                out=wp_stage, in_=wp_d.ap().rearrange("(cc p) d -> p cc d", p=128)
            )
            wp = consts.tile([128, CC, C], F32)
            nc.vector.tensor_copy(out=_rw(wp), in_=wp_stage)
            bq = consts.tile([128, 6], F32)
            nc.gpsimd.dma_start(
                out=bq, in_=bq_d.ap().rearrange("(m p) -> p m", p=128)
            )
            bp_row = consts.tile([1, C], F32)
            nc.gpsimd.dma_start(
                out=bp_row, in_=bp_d.ap().rearrange("(o c) -> o c", o=1)
            )
            bp_r = consts.tile([1, C], F32)
            nc.vector.tensor_copy(out=_rw(bp_r), in_=bp_row)
            gns = consts.tile([128, CC], F32)
            nc.gpsimd.dma_start(
                out=gns, in_=gns_d.ap().rearrange("(cc p) -> p cc", p=128)
            )
            gnb = consts.tile([128, CC], F32)
            nc.gpsimd.dma_start(
                out=gnb, in_=gnb_d.ap().rearrange("(cc p) -> p cc", p=128)
            )
            ones_raw = consts.tile([128, 1], F32)
            nc.vector.memset(ones_raw, 1.0)
            ones_col = consts.tile([128, 1], F32)
            nc.vector.tensor_copy(out=_rw(ones_col), in_=ones_raw)
            eps_col = consts.tile([128, 1], F32)
            nc.vector.memset(eps_col, EPS)

            # ---- phase A: transpose x to channel-major; bn_stats interleaved
            # so the statistics finish right after the last transpose ----
            stats = gn_stats.tile([128, CC, 8, 6], F32)
            for s in range(8):
                for nt in range(4 * s, 4 * s + 4):
                    for cc in range(CC):
                        ps = psum_tr.tile([128, 128], F32, tag="tr")
                        nc.tensor.transpose(
                            ps, x_tm[:, nt, cc * 128 : (cc + 1) * 128], ident
                        )
                        # alternate PSUM->SBUF copies across DVE and ACT so
                        # neither engine serializes the prologue
                        ceng = nc.vector if (nt + cc) % 2 == 0 else nc.scalar
                        if ceng is nc.vector:
                            ceng.tensor_copy(
                                out=_rw(x_cm[:, cc, nt * 128 : (nt + 1) * 128]),
                                in_=ps,
                            )
                        else:
                            nc.scalar.copy(
                                out=_rw(x_cm[:, cc, nt * 128 : (nt + 1) * 128]),
                                in_=ps,
                            )
                for cc in range(CC):
                    nc.vector.bn_stats(
                        out=stats[:, cc, s, :],
                        in_=x_cm[:, cc, s * 512 : (s + 1) * 512],
                    )

            # ---- groupnorm stats -> per-channel affine (a, b) ----
            ab = gn_stats.tile([128, CC, 2], F32)  # (a, b) per channel
            for cc in range(CC):
                mv = gn_stats.tile([128, 2], F32, tag="mv")
                nc.vector.bn_aggr(out=mv, in_=stats[:, cc, :, :])
                # mv2 = (mean, E[x^2])
                mv2 = gn_stats.tile([128, 2], F32, tag="mv2")
                nc.vector.tensor_copy(out=mv2[:, 0:1], in_=mv[:, 0:1])
                nc.vector.tensor_mul(out=mv2[:, 1:2], in0=mv[:, 0:1], in1=mv[:, 0:1])
                nc.vector.tensor_add(out=mv2[:, 1:2], in0=mv2[:, 1:2], in1=mv[:, 1:2])
                # aggregate to 16 group rows, then broadcast back to channels
                gp = psum_tr.tile([16, 2], F32, tag="tr", name="gp")
                nc.tensor.matmul(gp, lhsT=gA, rhs=mv2, start=True, stop=True)
                gp_sb = gn_stats.tile([16, 2], F32, tag="gp_sb")
                nc.vector.tensor_copy(out=gp_sb, in_=gp)
                chs = psum_tr.tile([128, 2], F32, tag="tr", name="chs")
                nc.tensor.matmul(chs, lhsT=gB, rhs=gp_sb, start=True, stop=True)
                chs_sb = gn_stats.tile([128, 2], F32, tag="chs_sb")
                nc.vector.tensor_copy(out=chs_sb, in_=chs)
                # var = E[x^2] - mean^2 ; rstd = 1/sqrt(var+eps)
                var = gn_stats.tile([128, 1], F32, tag="var")
                msq = gn_stats.tile([128, 1], F32, tag="msq")
                nc.vector.tensor_mul(out=msq, in0=chs_sb[:, 0:1], in1=chs_sb[:, 0:1])
                nc.vector.tensor_sub(out=var, in0=chs_sb[:, 1:2], in1=msq)
                nc.scalar.activation(
                    out=var,
                    in_=var,
                    func=mybir.ActivationFunctionType.Sqrt,
                    bias=eps_col,
                )
                rstd = gn_stats.tile([128, 1], F32, tag="rstd")
                nc.vector.reciprocal(out=rstd, in_=var)
                # a = rstd*gn_scale ; b = gn_bias - mean*a
                nc.vector.tensor_mul(
                    out=ab[:, cc, 0:1], in0=rstd, in1=gns[:, cc : cc + 1]
                )
                nc.vector.tensor_mul(out=msq, in0=chs_sb[:, 0:1], in1=ab[:, cc, 0:1])
                nc.vector.tensor_sub(
                    out=ab[:, cc, 1:2], in0=gnb[:, cc : cc + 1], in1=msq
                )

            # ---- fold the affine into the qkv weights:
            # qkv^T = (w*a)^T x^T + (w^T b + b_qkv) ----
            wq = consts.tile([128, CC, 3 * C], F32)
            for m in range(6):
                for cc in range(CC):
                    nc.scalar.mul(
                        out=_rw(wq[:, cc, m * 128 : (m + 1) * 128]),
                        in_=wq_stage[:, cc, m * 128 : (m + 1) * 128],
                        mul=ab[:, cc, 0:1],
                    )
            bias2 = gn_stats.tile([128, 6], F32)
            for m in range(6):
                psb = psum_tr.tile([128, 1], F32, tag="tr", name="psb")
                for cc in range(CC):
                    nc.tensor.matmul(
                        psb,
                        lhsT=wq_stage[:, cc, m * 128 : (m + 1) * 128],
                        rhs=ab[:, cc, 1:2],
                        start=(cc == 0),
                        stop=(cc == CC - 1),
                    )
                nc.vector.tensor_add(
                    out=bias2[:, m : m + 1], in0=psb, in1=bq[:, m : m + 1]
                )

            # ---- phase B: qkv^T = wq.T @ x^T (+ bias2) ----
            for m in range(6):
                for qt in range(NQ):
                    ps = psum_mm.tile([128, QT], F32, tag="mm")
                    for cc in range(CC):
                        nc.tensor.matmul(
                            ps,
                            lhsT=_mm(wq[:, cc, m * 128 : (m + 1) * 128]),
                            rhs=_mm(x_cm[:, cc, qt * QT : (qt + 1) * QT]),
                            start=(cc == 0),
                            stop=(cc == CC - 1),
                        )
                    if qt % 2 == 0:
                        nc.scalar.activation(
                            out=_rw(qkvT[:, m, qt * QT : (qt + 1) * QT]),
                            in_=ps,
                            func=mybir.ActivationFunctionType.Identity,
                            bias=bias2[:, m : m + 1],
                        )
                    else:
                        nc.vector.tensor_scalar_add(
                            out=_rw(qkvT[:, m, qt * QT : (qt + 1) * QT]),
                            in0=ps,
                            scalar1=bias2[:, m : m + 1],
                        )

        # ---- phase C: V token-major via PE transposes ----
        with tc.tile_pool(name="vtm", bufs=1) as vtm_pool:
            v_tm = vtm_pool.tile([128, 32, C], F32)
            for nt in range(32):
                for cc in range(CC):
                    ps = psum_tr.tile([128, 128], F32, tag="tr")
                    nc.tensor.transpose(
                        ps, qkvT[:, 4 + cc, nt * 128 : (nt + 1) * 128], ident
                    )
                    nc.vector.tensor_copy(
                        out=_rw(v_tm[:, nt, cc * 128 : (cc + 1) * 128]), in_=ps
                    )

            # ---- phase D: attention + proj + skip, per q tile ----
            with (
                tc.tile_pool(name="expp", bufs=6) as expp,
                tc.tile_pool(name="accp", bufs=2) as accp,
                tc.tile_pool(name="owork", bufs=2) as owork,
            ):
                def emit_lg(qt, kt):
                    lg = psum_mm.tile([128, QT], F32, tag="mm", name="lg")
                    for cc in range(CC):
                        nc.tensor.matmul(
                            lg,
                            lhsT=_mm(qkvT[:, 2 + cc, kt * 128 : (kt + 1) * 128]),
                            rhs=_mm(qkvT[:, cc, qt * QT : (qt + 1) * QT]),
                            start=(cc == 0),
                            stop=(cc == CC - 1),
                        )
                    return lg

                # logits tiles prefetched across the qt boundary
                next_lgs = {kk: emit_lg(0, kk) for kk in range(2)}
                for qt in range(NQ):
                    av_ps = [
                        psum_acc.tile(
                            [128, QT], F32, tag=f"av_ps{cc}", name=f"av_ps{cc}"
                        )
                        for cc in range(CC)
                    ]
                    expacc = accp.tile([128, QT], F32, tag="expacc")
                    expacc2 = accp.tile([128, QT], F32, tag="expacc2")

                    def emit_av(kt, expT):
                        for cc in range(CC):
                            nc.tensor.matmul(
                                av_ps[cc],
                                lhsT=_mm(v_tm[:, kt, cc * 128 : (cc + 1) * 128]),
                                rhs=_mm(expT),
                                start=(kt == 0),
                                stop=(kt == NK - 1),
                            )

                    # software pipeline, kt unrolled by 2: the PE stays 4+
                    # logits-matmuls ahead of each av, fully hiding the
                    # lg -> exp(ACT) -> av semaphore+latency chain (~910ns).
                    lgs = next_lgs
                    lgs[2] = emit_lg(qt, 2)
                    lgs[3] = emit_lg(qt, 3)
                    for kt0 in range(0, NK, 2):
                        expTs = {}
                        for j in (kt0, kt0 + 1):
                            lg = lgs.pop(j)
                            expT = expp.tile([128, QT], F32, tag="expT")
                            nc.scalar.activation(
                                out=_rw(expT),
                                in_=lg,
                                func=mybir.ActivationFunctionType.Exp,
                                scale=1.0 / 16.0,
                            )
                            expTs[j] = expT
                            # softmax-denominator accumulation on the DVE,
                            # two alternating accumulators to halve the
                            # serial dependence
                            eng = nc.vector
                            acc = expacc2 if j % 2 == 0 else expacc
                            if j < 2:
                                eng.tensor_copy(out=acc, in_=expT)
                            else:
                                eng.tensor_add(out=acc, in0=acc, in1=expT)
                        for j in (kt0 + 4, kt0 + 5):
                            if j < NK:
                                lgs[j] = emit_lg(qt, j)
                        for j in (kt0, kt0 + 1):
                            emit_av(j, expTs[j])

                    # prefetch the next q tile's first logits so the PE
                    # stays busy while the colsum/proj chain drains
                    if qt + 1 < NQ:
                        next_lgs = {kk: emit_lg(qt + 1, kk) for kk in range(2)}
                    expcomb = accp.tile([128, QT], F32, tag="expcomb")
                    nc.vector.tensor_add(
                        out=_rw(expcomb), in0=expacc, in1=expacc2
                    )
                    # softmax denominator: colsum = ones.T @ expcomb
                    cs = psum_tr.tile([1, QT], F32, tag="tr", name="cs")
                    nc.tensor.matmul(
                        cs, lhsT=_mm(ones_col), rhs=_mm(expcomb), start=True, stop=True
                    )
                    cs_sb = owork.tile([1, QT], F32, tag="cs_sb")
                    nc.vector.tensor_copy(out=_rw(cs_sb), in_=cs)
                    cs_dram = dram_scratch.tile([QT], F32)
                    nc.sync.dma_start(out=cs_dram, in_=cs_sb)
                    recip = owork.tile([128, 4], F32, tag="recip")
                    nc.sync.dma_start(
                        out=recip, in_=cs_dram.rearrange("(qq p) -> p qq", p=128)
                    )
                    nc.vector.reciprocal(out=recip, in_=recip)

                    # proj_un^T = w_proj.T @ o_un^T + b_proj x colsum
                    av_sb = owork.tile([128, CC, QT], F32, tag="av_sb")
                    for cc in range(CC):
                        nc.vector.tensor_copy(out=_rw(av_sb[:, cc, :]), in_=av_ps[cc])
                    pj_sb = owork.tile([128, CC, QT], F32, tag="pj_sb")
                    for dc in range(CC):
                        ps = psum_mm.tile([128, QT], F32, tag="mm", name="pj_ps")
                        for cc in range(CC):
                            nc.tensor.matmul(
                                ps,
                                lhsT=_mm(wp[:, cc, dc * 128 : (dc + 1) * 128]),
                                rhs=_mm(av_sb[:, cc, :]),
                                start=(cc == 0),
                                stop=False,
                            )
                        nc.tensor.matmul(
                            ps,
                            lhsT=_mm(bp_r[:, dc * 128 : (dc + 1) * 128]),
                            rhs=_mm(cs_sb),
                            start=False,
                            stop=True,
                        )
                        nc.scalar.copy(out=pj_sb[:, dc, :], in_=ps)

                    # back to token-major; apply 1/colsum; add skip
                    out_sb = owork.tile([128, 4, C], F32, tag="out_sb")
                    x_re = owork.tile([128, 4, C], F32, tag="x_re")
                    nc.sync.dma_start(
                        out=x_re, in_=x_tok[:, qt * 4 : (qt + 1) * 4, :]
                    )
                    for qq in range(4):
                        for dc in range(CC):
                            ps = psum_tr.tile([128, 128], F32, tag="tr", name="ps_out")
                            nc.tensor.transpose(
                                ps, pj_sb[:, dc, qq * 128 : (qq + 1) * 128], ident
                            )
                            nc.vector.tensor_scalar_mul(
                                out=out_sb[:, qq, dc * 128 : (dc + 1) * 128],
                                in0=ps,
                                scalar1=recip[:, qq : qq + 1],
                            )
                    nc.vector.tensor_add(out=out_sb, in0=out_sb, in1=x_re)
                    nc.sync.dma_start(
                        out=out_tok[:, qt * 4 : (qt + 1) * 4, :], in_=out_sb
                    )


_NC = None


def _get_nc():
    global _NC
    if _NC is None:
        _NC = _build()
    return _NC


_RUNNER = None

IN_NAMES = ["x", "gn_scale", "gn_bias", "w_qkv", "b_qkv", "w_proj", "b_proj"]


def _get_runner():
    """Cached jitted shard_map executable over the 8 cores (the equivalent of
    run_bass_kernel_spmd's axon path, but built once instead of per call)."""
    global _RUNNER
    if _RUNNER is not None:
        return _RUNNER
    import jax
    from jax.sharding import Mesh, PartitionSpec
    from jax.experimental.shard_map import shard_map
    from concourse import bass2jax

    nc = _get_nc()
    bass2jax.install_neuronx_cc_hook()

    in_names = list(IN_NAMES) + ["out"]
    if nc.partition_id_tensor is not None:
        in_names.append(nc.partition_id_tensor.name)

    def _body_fn(*args):
        operands = list(args)
        if nc.partition_id_tensor is not None:
            operands.append(bass2jax.partition_id_tensor())
        outs = bass2jax._bass_exec_p.bind(
            *operands,
            out_avals=(jax.core.ShapedArray((N, C), np.float32),),
            in_names=tuple(in_names),
            out_names=("out",),
            lowering_input_output_aliases=(),
            sim_require_finite=True,
            sim_require_nnan=True,
            nc=nc,
        )
        return tuple(outs)

    devices = jax.devices()[:N_CORES]
    mesh = Mesh(np.asarray(devices), ("core",))
    in_specs = (PartitionSpec("core"),) * (len(IN_NAMES) + 1)
    out_specs = (PartitionSpec("core"),)
    sharded = jax.jit(
        shard_map(
            _body_fn, mesh=mesh, in_specs=in_specs, out_specs=out_specs,
            check_rep=False,
        ),
        donate_argnums=(len(IN_NAMES),),
        keep_unused=True,
    )
    _RUNNER = sharded
    return _RUNNER


def kernel(x, gn_scale, gn_bias, w_qkv, b_qkv, w_proj, b_proj):
    sharded = _get_runner()
    x = np.ascontiguousarray(np.asarray(x, dtype=np.float32).reshape(B * N, C))
    shared = {
        "gn_scale": np.asarray(gn_scale, np.float32),
        "gn_bias": np.asarray(gn_bias, np.float32),
        "w_qkv": np.ascontiguousarray(np.asarray(w_qkv, np.float32)),
        "b_qkv": np.asarray(b_qkv, np.float32),
        "w_proj": np.ascontiguousarray(np.asarray(w_proj, np.float32)),
        "b_proj": np.asarray(b_proj, np.float32),
    }
    # shard_map slices axis 0 across cores: x gets its own batch; the shared
    # weights are tiled 8x so every core sees an identical copy.
    concat = [x]
    for name in IN_NAMES[1:]:
        a = shared[name]
        concat.append(np.concatenate([a] * N_CORES, axis=0))
    zeros = np.zeros((N_CORES * N, C), np.float32)
    (out,) = sharded(*concat, zeros)
    return np.asarray(out).reshape(B, H, W, C)
